# revision 9
# baseline (speedup 1.0000x reference)
"""Trainium2 Bass kernel v9: cross-entropy with Gaussian-smoothed labels.

loss = mean over tokens of [ wsum(t) * logsumexp(pred_row) - sum_k w_k * pred[win_k] ]

Key ideas vs v8 (which spent ~90us on 64 indirect-DMA gathers):
  - The reference's scatter-with-clamp smoothed label reduces exactly to
    W[t,c] = g(|c - tgt_t|), g = [1, e^-.5, e^-1, e^-2], 0 beyond +-3
    (last-write-wins makes the smallest distance win at the boundaries).
  - The loss is a mean over tokens => permutation invariant. Host sorts
    tokens by target, so each group of 4 tiles (512 sorted tokens) has all
    its 7-wide windows inside one static 80-wide class band. The windowed
    term becomes 16 rectangular bf16 multiplies against host-shipped exact
    band weights; zero indirect DMAs.
  - rel-err gate is 2e-2: stream pred in bf16 (11.8 MB/core instead of
    23.7), measured end-to-end error ~1e-5.
  - per-token sum(exp) split between ACT (exp + HW accumulator per token
    column) and DVE (batched exp + reduce) so both engines finish together
    (~46us each); DVE TensorReduce has no 2x bf16 mode so ACT takes ~20
    columns.
  - per-core partial sums [128,1] f32 DMA'd out; host sums in f64.
"""
import math

import numpy as np
import ml_dtypes

import concourse.bass as bass
import concourse.bacc as bacc
import concourse.tile as tile
from concourse import mybir
from concourse import bass_utils

B, T, C = 32, 2048, 722
CORES = 8
SHARD = B * T // CORES          # 8192 tokens per core
P = 128
TILES = SHARD // P              # 64 token tiles of 128
G = 8                           # tiles per stream group (one DMA each)
NG = TILES // G                 # 8 stream groups
GB = 4                          # tiles per band group (shared class band)
NGB = TILES // GB               # 16 band groups
WIDTH = 80                      # static band width (seed-0 data needs 65)
DECAYS = [math.exp(-(2.0 ** d) / 4.0) for d in range(4)]
# accum columns per stream group (ACT-side sum(exp)); rest go DVE reduce.
# group 0 is split 4+4 for an early ACT start, so it gets none.
NA = (0, 2, 2, 2, 2, 2, 2, 2)

BF16 = ml_dtypes.bfloat16

_ALU = mybir.AluOpType
_ACT = mybir.ActivationFunctionType

# g(d) lookup, exact reference decay values (g(0)=1 from the final set())
_GVAL = np.zeros(8, np.float32)
for _d in range(4):
    _GVAL[_d] = 1.0 if _d == 0 else DECAYS[_d]

_NC_CACHE = {}


def _build(band_starts, pred_bufs=4, exp_bufs=3):
    nc = bacc.Bacc("TRN2", target_bir_lowering=False, debug=False,
                   enable_asserts=False, num_devices=CORES)
    pred = nc.dram_tensor("pred", [SHARD, C], mybir.dt.bfloat16,
                          kind="ExternalInput")
    wband = nc.dram_tensor("wband", [P, TILES * WIDTH], mybir.dt.bfloat16,
                           kind="ExternalInput")
    wsum_in = nc.dram_tensor("wsum", [P, TILES], mybir.dt.float32,
                             kind="ExternalInput")
    out = nc.dram_tensor("partial", [P, 1], mybir.dt.float32,
                         kind="ExternalOutput")

    # HBM row r = jg*(P*G) + p*G + g holds token sorted[(jg*G+g)*P + p]:
    # per-partition contiguous G rows -> one clean 11.5KB descriptor set.
    pred_r = pred.ap().rearrange("(j p g) c -> j p g c", p=P, g=G)

    with tile.TileContext(nc) as tc:
        with (tc.tile_pool(name="pred", bufs=pred_bufs) as pred_pool,
              tc.tile_pool(name="exp", bufs=exp_bufs) as exp_pool,
              tc.tile_pool(name="small", bufs=1) as small):
            W_sb = small.tile([P, TILES, WIDTH], mybir.dt.bfloat16)
            wsum_sb = small.tile([P, TILES], mybir.dt.float32)
            sums = small.tile([P, TILES], mybir.dt.float32)
            m = small.tile([P, TILES, WIDTH], mybir.dt.bfloat16)
            gsum = small.tile([P, TILES], mybir.dt.float32)
            junk = small.tile([P, C], mybir.dt.bfloat16)
            junk1 = small.tile([P, 1], mybir.dt.float32)

            def process(j0, pt, nt, na):
                """exp+sums+bands for nt tiles at global column j0, held in
                pt[:, :nt, :]. na trailing columns use the ACT accumulator.
                nt must be a multiple of GB and j0 GB-aligned."""
                nb = nt - na
                # batched exp first so the DVE reduce starts early; HW
                # accumulator paired with a junk read (proven v8 pattern).
                et = exp_pool.tile([P, G, C], mybir.dt.bfloat16)
                nc.scalar.activation(out=et[:, :nb, :], in_=pt[:, :nb, :],
                                     func=_ACT.Exp, accum_out=junk1)
                nc.vector.reduce_sum(out=sums[:, j0:j0 + nb],
                                     in_=et[:, :nb, :],
                                     axis=mybir.AxisListType.X)
                for g in range(na):
                    j = j0 + nb + g
                    nc.scalar.activation(out=junk, in_=pt[:, nb + g, :],
                                         func=_ACT.Exp,
                                         accum_out=sums[:, j:j + 1])
                # windowed term (gpsimd tensor ops NaN on HW; DVE takes it)
                for b in range(nt // GB):
                    gi = (j0 + b * GB) // GB
                    s = band_starts[gi]
                    nc.vector.tensor_mul(
                        out=m[:, j0 + b * GB:j0 + (b + 1) * GB, :],
                        in0=pt[:, b * GB:(b + 1) * GB, s:s + WIDTH],
                        in1=W_sb[:, j0 + b * GB:j0 + (b + 1) * GB, :])
                nc.vector.reduce_sum(out=gsum[:, j0:j0 + nt],
                                     in_=m[:, j0:j0 + nt, :],
                                     axis=mybir.AxisListType.X)

            # group 0 split 4+4: first exp starts ~10us earlier than one
            # 1.5MB DMA would allow. W/wsum ride the sync ring behind it
            # (the stream has ~20us of slack; ACT is the critical engine).
            pt0a = pred_pool.tile([P, G, C], mybir.dt.bfloat16)
            pt0b = pred_pool.tile([P, G, C], mybir.dt.bfloat16)
            nc.sync.dma_start(out=pt0a[:, :GB, :], in_=pred_r[0][:, :GB, :])
            nc.sync.dma_start(out=pt0b[:, :GB, :], in_=pred_r[0][:, GB:, :])
            nc.sync.dma_start(
                out=W_sb,
                in_=wband.ap().rearrange("p (j w) -> p j w", w=WIDTH))
            nc.sync.dma_start(out=wsum_sb, in_=wsum_in.ap())
            process(0, pt0a, GB, 0)
            process(GB, pt0b, GB, 0)

            for jg in range(1, NG):
                pt = pred_pool.tile([P, G, C], mybir.dt.bfloat16)
                nc.sync.dma_start(out=pt, in_=pred_r[jg])
                process(jg * G, pt, G, NA[jg])

            # tail: ~1.5us
            lse = small.tile([P, TILES], mybir.dt.float32)
            loss = small.tile([P, TILES], mybir.dt.float32)
            part = small.tile([P, 1], mybir.dt.float32)
            nc.scalar.activation(out=lse, in_=sums, func=_ACT.Ln)
            nc.vector.tensor_mul(out=loss, in0=wsum_sb, in1=lse)
            nc.vector.tensor_sub(out=loss, in0=loss, in1=gsum)
            nc.vector.reduce_sum(out=part, in_=loss,
                                 axis=mybir.AxisListType.X)
            nc.scalar.dma_start(out=out.ap(), in_=part)
    nc.compile()
    return nc


def _get_nc(band_starts):
    key = tuple(band_starts)
    if key not in _NC_CACHE:
        _NC_CACHE[key] = _build(key)
    return _NC_CACHE[key]


def _band_starts(target):
    """Static per-band-group class-band starts, shared by all cores."""
    lo = np.full(NGB, 1 << 30, np.int64)
    hi = np.full(NGB, -1, np.int64)
    bpc = B // CORES
    for c in range(CORES):
        tg = np.sort(target[c * bpc:(c + 1) * bpc].reshape(-1))
        blocks = tg.reshape(NGB, GB * P)
        lo = np.minimum(lo, np.clip(blocks.min(axis=1) - 3, 0, C - 1))
        hi = np.maximum(hi, np.clip(blocks.max(axis=1) + 3, 0, C - 1))
    assert (hi - lo + 1).max() <= WIDTH, "band width exceeded"
    s = np.clip((lo + hi + 1 - WIDTH) // 2, 0, C - WIDTH)
    assert np.all((lo >= s) & (hi < s + WIDTH))
    return tuple(int(x) for x in s)


def _shard_inputs(pred, target, band_starts):
    bpc = B // CORES
    s_per_tile = np.asarray(band_starts, np.int64)[np.arange(TILES) // GB]
    i_idx = np.arange(WIDTH)
    in_maps = []
    for c in range(CORES):
        shard_pred = pred[c * bpc:(c + 1) * bpc].reshape(SHARD, C)
        tg = np.ascontiguousarray(
            target[c * bpc:(c + 1) * bpc].reshape(SHARD)).astype(np.int64)
        order = np.argsort(tg, kind="stable")
        # HBM row r = jg*(P*G) + p*G + g  <->  token order[(jg*G+g)*P + p]
        r = np.arange(SHARD)
        jgr, rem = np.divmod(r, P * G)
        pr, gr = np.divmod(rem, G)
        perm = order[(jgr * G + gr) * P + pr]
        pred_rows = np.ascontiguousarray(shard_pred[perm]).astype(BF16)
        # W[p, j, i] = g(|band_col - tgt|), exact reference weights
        tgt_pj = tg[order].reshape(TILES, P).T              # (P, TILES)
        cpos = s_per_tile[None, :, None] + i_idx[None, None, :]
        d = np.abs(cpos - tgt_pj[:, :, None])               # (P, TILES, W)
        W = _GVAL[np.minimum(d, 7)]                         # f32 exact
        wsum = W.sum(axis=2, dtype=np.float32)              # (P, TILES) f32
        in_maps.append({
            "pred": pred_rows,
            "wband": np.ascontiguousarray(W.astype(BF16).reshape(P, -1)),
            "wsum": np.ascontiguousarray(wsum),
        })
    return in_maps


def _run(pred, target, **kwargs):
    pred = np.asarray(pred)
    target = np.asarray(target)
    band_starts = _band_starts(target)
    nc = _get_nc(band_starts)
    return bass_utils.run_bass_kernel_spmd(
        nc, _shard_inputs(pred, target, band_starts),
        core_ids=list(range(CORES)), **kwargs)


def kernel(pred, target):
    res = _run(pred, target)
    total = sum(float(r["partial"].astype(np.float64).sum())
                for r in res.results)
    return np.asarray(total / (B * T), dtype=np.float32)


# revision 10
# speedup vs baseline: 1.3266x; 1.3266x over previous
"""Trainium2 Bass kernel v9: cross-entropy with Gaussian-smoothed labels.

loss = mean over tokens of [ wsum(t) * logsumexp(pred_row) - sum_k w_k * pred[win_k] ]

Key ideas vs v8 (which spent ~90us on 64 indirect-DMA gathers):
  - The reference's scatter-with-clamp smoothed label reduces exactly to
    W[t,c] = g(|c - tgt_t|), g = [1, e^-.5, e^-1, e^-2], 0 beyond +-3
    (last-write-wins makes the smallest distance win at the boundaries).
  - The loss is a mean over tokens => permutation invariant. Host sorts
    tokens by target, so each group of 4 tiles (512 sorted tokens) has all
    its 7-wide windows inside one static 80-wide class band. The windowed
    term becomes 16 rectangular bf16 multiplies against host-shipped exact
    band weights; zero indirect DMAs.
  - rel-err gate is 2e-2: stream pred in bf16 (11.8 MB/core instead of
    23.7), measured end-to-end error ~1e-5.
  - per-token sum(exp) split between ACT (exp + HW accumulator per token
    column) and DVE (batched exp + reduce) so both engines finish together
    (~46us each); DVE TensorReduce has no 2x bf16 mode so ACT takes ~20
    columns.
  - per-core partial sums [128,1] f32 DMA'd out; host sums in f64.
"""
import math

import numpy as np
import ml_dtypes

import concourse.bass as bass
import concourse.bacc as bacc
import concourse.tile as tile
from concourse import mybir
from concourse import bass_utils

B, T, C = 32, 2048, 722
CORES = 8
SHARD = B * T // CORES          # 8192 tokens per core
P = 128
TILES = SHARD // P              # 64 token tiles of 128
G = 8                           # tiles per stream group (one DMA each)
NG = TILES // G                 # 8 stream groups
GB = 4                          # tiles per band group (shared class band)
NGB = TILES // GB               # 16 band groups
WIDTH = 80                      # static band width (seed-0 data needs 65)
DECAYS = [math.exp(-(2.0 ** d) / 4.0) for d in range(4)]
# accum columns per stream group (ACT-side sum(exp)); rest go DVE reduce.
# group 0 is split 4+4 for an early ACT start, so it gets none.
NA = (0, 2, 2, 2, 2, 2, 2, 2)

BF16 = ml_dtypes.bfloat16

_ALU = mybir.AluOpType
_ACT = mybir.ActivationFunctionType

# g(d) lookup, exact reference decay values (g(0)=1 from the final set())
_GVAL = np.zeros(8, np.float32)
for _d in range(4):
    _GVAL[_d] = 1.0 if _d == 0 else DECAYS[_d]

_NC_CACHE = {}


def _build(band_starts, pred_bufs=4, exp_bufs=3):
    nc = bacc.Bacc("TRN2", target_bir_lowering=False, debug=False,
                   enable_asserts=False, num_devices=CORES)
    pred = nc.dram_tensor("pred", [SHARD, C], mybir.dt.bfloat16,
                          kind="ExternalInput")
    wband = nc.dram_tensor("wband", [P, TILES * WIDTH], mybir.dt.bfloat16,
                           kind="ExternalInput")
    wsum_in = nc.dram_tensor("wsum", [P, TILES], mybir.dt.float32,
                             kind="ExternalInput")
    out = nc.dram_tensor("partial", [P, 1], mybir.dt.float32,
                         kind="ExternalOutput")

    # HBM row r = jg*(P*G) + p*G + g holds token sorted[(jg*G+g)*P + p]:
    # per-partition contiguous G rows -> one clean 11.5KB descriptor set.
    pred_r = pred.ap().rearrange("(j p g) c -> j p g c", p=P, g=G)

    with tile.TileContext(nc) as tc:
        with (tc.tile_pool(name="pred", bufs=pred_bufs) as pred_pool,
              tc.tile_pool(name="exp", bufs=exp_bufs) as exp_pool,
              tc.tile_pool(name="small", bufs=1) as small):
            W_sb = small.tile([P, TILES, WIDTH], mybir.dt.bfloat16)
            wsum_sb = small.tile([P, TILES], mybir.dt.float32)
            sums = small.tile([P, TILES], mybir.dt.float32)
            m = small.tile([P, TILES, WIDTH], mybir.dt.bfloat16)
            gsum = small.tile([P, TILES], mybir.dt.float32)
            junk = small.tile([P, C], mybir.dt.bfloat16)
            junk1 = small.tile([P, 1], mybir.dt.float32)

            def process(j0, pt, nt, na):
                """exp+sums+bands for nt tiles at global column j0, held in
                pt[:, :nt, :]. na trailing columns use the ACT accumulator.
                nt must be a multiple of GB and j0 GB-aligned."""
                nb = nt - na
                # batched exp first so the DVE reduce starts early; HW
                # accumulator paired with a junk read (proven v8 pattern).
                et = exp_pool.tile([P, G, C], mybir.dt.bfloat16)
                nc.scalar.activation(out=et[:, :nb, :], in_=pt[:, :nb, :],
                                     func=_ACT.Exp, accum_out=junk1)
                nc.vector.reduce_sum(out=sums[:, j0:j0 + nb],
                                     in_=et[:, :nb, :],
                                     axis=mybir.AxisListType.X)
                for g in range(na):
                    j = j0 + nb + g
                    nc.scalar.activation(out=junk, in_=pt[:, nb + g, :],
                                         func=_ACT.Exp,
                                         accum_out=sums[:, j:j + 1])
                # windowed term (gpsimd tensor ops NaN on HW; DVE takes it)
                for b in range(nt // GB):
                    gi = (j0 + b * GB) // GB
                    s = band_starts[gi]
                    nc.vector.tensor_mul(
                        out=m[:, j0 + b * GB:j0 + (b + 1) * GB, :],
                        in0=pt[:, b * GB:(b + 1) * GB, s:s + WIDTH],
                        in1=W_sb[:, j0 + b * GB:j0 + (b + 1) * GB, :])
                nc.vector.reduce_sum(out=gsum[:, j0:j0 + nt],
                                     in_=m[:, j0:j0 + nt, :],
                                     axis=mybir.AxisListType.X)

            # group 0 split 4+4: first exp starts ~10us earlier than one
            # 1.5MB DMA would allow. W/wsum ride the sync ring behind it
            # (the stream has ~20us of slack; ACT is the critical engine).
            pt0a = pred_pool.tile([P, G, C], mybir.dt.bfloat16)
            pt0b = pred_pool.tile([P, G, C], mybir.dt.bfloat16)
            nc.sync.dma_start(out=pt0a[:, :GB, :], in_=pred_r[0][:, :GB, :])
            nc.sync.dma_start(out=pt0b[:, :GB, :], in_=pred_r[0][:, GB:, :])
            # W/wsum ride the scalar ring in halves so the sync ring carries
            # only pred; the first half covers the early groups' band mults
            wband_v = wband.ap().rearrange("p (j w) -> p j w", w=WIDTH)
            nc.scalar.dma_start(out=wsum_sb, in_=wsum_in.ap())
            nc.scalar.dma_start(out=W_sb[:, :TILES // 2, :],
                                in_=wband_v[:, :TILES // 2, :])
            nc.scalar.dma_start(out=W_sb[:, TILES // 2:, :],
                                in_=wband_v[:, TILES // 2:, :])
            process(0, pt0a, GB, 0)
            process(GB, pt0b, GB, 0)

            for jg in range(1, NG):
                pt = pred_pool.tile([P, G, C], mybir.dt.bfloat16)
                nc.sync.dma_start(out=pt, in_=pred_r[jg])
                process(jg * G, pt, G, NA[jg])

            # tail: ~1.5us
            lse = small.tile([P, TILES], mybir.dt.float32)
            loss = small.tile([P, TILES], mybir.dt.float32)
            part = small.tile([P, 1], mybir.dt.float32)
            nc.scalar.activation(out=lse, in_=sums, func=_ACT.Ln)
            nc.vector.tensor_mul(out=loss, in0=wsum_sb, in1=lse)
            nc.vector.tensor_sub(out=loss, in0=loss, in1=gsum)
            nc.vector.reduce_sum(out=part, in_=loss,
                                 axis=mybir.AxisListType.X)
            nc.scalar.dma_start(out=out.ap(), in_=part)
    nc.compile()
    return nc


def _get_nc(band_starts):
    key = tuple(band_starts)
    if key not in _NC_CACHE:
        _NC_CACHE[key] = _build(key)
    return _NC_CACHE[key]


def _band_starts(target):
    """Static per-band-group class-band starts, shared by all cores."""
    lo = np.full(NGB, 1 << 30, np.int64)
    hi = np.full(NGB, -1, np.int64)
    bpc = B // CORES
    for c in range(CORES):
        tg = np.sort(target[c * bpc:(c + 1) * bpc].reshape(-1))
        blocks = tg.reshape(NGB, GB * P)
        lo = np.minimum(lo, np.clip(blocks.min(axis=1) - 3, 0, C - 1))
        hi = np.maximum(hi, np.clip(blocks.max(axis=1) + 3, 0, C - 1))
    assert (hi - lo + 1).max() <= WIDTH, "band width exceeded"
    s = np.clip((lo + hi + 1 - WIDTH) // 2, 0, C - WIDTH)
    assert np.all((lo >= s) & (hi < s + WIDTH))
    return tuple(int(x) for x in s)


def _shard_inputs(pred, target, band_starts):
    bpc = B // CORES
    s_per_tile = np.asarray(band_starts, np.int64)[np.arange(TILES) // GB]
    i_idx = np.arange(WIDTH)
    in_maps = []
    for c in range(CORES):
        shard_pred = pred[c * bpc:(c + 1) * bpc].reshape(SHARD, C)
        tg = np.ascontiguousarray(
            target[c * bpc:(c + 1) * bpc].reshape(SHARD)).astype(np.int64)
        order = np.argsort(tg, kind="stable")
        # HBM row r = jg*(P*G) + p*G + g  <->  token order[(jg*G+g)*P + p]
        r = np.arange(SHARD)
        jgr, rem = np.divmod(r, P * G)
        pr, gr = np.divmod(rem, G)
        perm = order[(jgr * G + gr) * P + pr]
        pred_rows = np.ascontiguousarray(shard_pred[perm]).astype(BF16)
        # W[p, j, i] = g(|band_col - tgt|), exact reference weights
        tgt_pj = tg[order].reshape(TILES, P).T              # (P, TILES)
        cpos = s_per_tile[None, :, None] + i_idx[None, None, :]
        d = np.abs(cpos - tgt_pj[:, :, None])               # (P, TILES, W)
        W = _GVAL[np.minimum(d, 7)]                         # f32 exact
        wsum = W.sum(axis=2, dtype=np.float32)              # (P, TILES) f32
        in_maps.append({
            "pred": pred_rows,
            "wband": np.ascontiguousarray(W.astype(BF16).reshape(P, -1)),
            "wsum": np.ascontiguousarray(wsum),
        })
    return in_maps


def _run(pred, target, **kwargs):
    pred = np.asarray(pred)
    target = np.asarray(target)
    band_starts = _band_starts(target)
    nc = _get_nc(band_starts)
    return bass_utils.run_bass_kernel_spmd(
        nc, _shard_inputs(pred, target, band_starts),
        core_ids=list(range(CORES)), **kwargs)


def kernel(pred, target):
    res = _run(pred, target)
    total = sum(float(r["partial"].astype(np.float64).sum())
                for r in res.results)
    return np.asarray(total / (B * T), dtype=np.float32)


# revision 13
# speedup vs baseline: 1.3568x; 1.0228x over previous
"""Trainium2 Bass kernel v9: cross-entropy with Gaussian-smoothed labels.

loss = mean over tokens of [ wsum(t) * logsumexp(pred_row) - sum_k w_k * pred[win_k] ]

Key ideas vs v8 (which spent ~90us on 64 indirect-DMA gathers):
  - The reference's scatter-with-clamp smoothed label reduces exactly to
    W[t,c] = g(|c - tgt_t|), g = [1, e^-.5, e^-1, e^-2], 0 beyond +-3
    (last-write-wins makes the smallest distance win at the boundaries).
  - The loss is a mean over tokens => permutation invariant. Host sorts
    tokens by target, so each group of 4 tiles (512 sorted tokens) has all
    its 7-wide windows inside one static 80-wide class band. The windowed
    term becomes 16 rectangular bf16 multiplies against host-shipped exact
    band weights; zero indirect DMAs.
  - rel-err gate is 2e-2: stream pred in bf16 (11.8 MB/core instead of
    23.7), measured end-to-end error ~1e-5.
  - per-token sum(exp) split between ACT (exp + HW accumulator per token
    column) and DVE (batched exp + reduce) so both engines finish together
    (~46us each); DVE TensorReduce has no 2x bf16 mode so ACT takes ~20
    columns.
  - per-core partial sums [128,1] f32 DMA'd out; host sums in f64.
"""
import math

import numpy as np
import ml_dtypes

import concourse.bass as bass
import concourse.bacc as bacc
import concourse.tile as tile
from concourse import mybir
from concourse import bass_utils

B, T, C = 32, 2048, 722
CORES = 8
SHARD = B * T // CORES          # 8192 tokens per core
P = 128
TILES = SHARD // P              # 64 token tiles of 128
G = 8                           # tiles per stream group (one DMA each)
NG = TILES // G                 # 8 stream groups
GB = 4                          # tiles per band group (shared class band)
NGB = TILES // GB               # 16 band groups
WIDTH = 80                      # static band width (seed-0 data needs 65)
DECAYS = [math.exp(-(2.0 ** d) / 4.0) for d in range(4)]
# accum columns per stream group (ACT-side sum(exp)); rest go DVE reduce.
# group 0 is split 4+4 for an early ACT start, so it gets none.
NA = (0, 2, 2, 2, 2, 2, 2, 2)

BF16 = ml_dtypes.bfloat16

_ALU = mybir.AluOpType
_ACT = mybir.ActivationFunctionType

# g(d) lookup, exact reference decay values (g(0)=1 from the final set())
_GVAL = np.zeros(8, np.float32)
for _d in range(4):
    _GVAL[_d] = 1.0 if _d == 0 else DECAYS[_d]

_NC_CACHE = {}


def _build(band_starts, pred_bufs=4, exp_bufs=3):
    nc = bacc.Bacc("TRN2", target_bir_lowering=False, debug=False,
                   enable_asserts=False, num_devices=CORES)
    pred = nc.dram_tensor("pred", [SHARD, C], mybir.dt.bfloat16,
                          kind="ExternalInput")
    wband = nc.dram_tensor("wband", [P, TILES * WIDTH], mybir.dt.bfloat16,
                           kind="ExternalInput")
    wsum_in = nc.dram_tensor("wsum", [P, TILES], mybir.dt.float32,
                             kind="ExternalInput")
    out = nc.dram_tensor("partial", [P, 1], mybir.dt.float32,
                         kind="ExternalOutput")

    # HBM row r = jg*(P*G) + p*G + g holds token sorted[(jg*G+g)*P + p]:
    # per-partition contiguous G rows -> one clean 11.5KB descriptor set.
    pred_r = pred.ap().rearrange("(j p g) c -> j p g c", p=P, g=G)

    with tile.TileContext(nc) as tc:
        with (tc.tile_pool(name="pred", bufs=pred_bufs) as pred_pool,
              tc.tile_pool(name="exp", bufs=exp_bufs) as exp_pool,
              tc.tile_pool(name="small", bufs=1) as small):
            W_sb = small.tile([P, TILES, WIDTH], mybir.dt.bfloat16)
            wsum_sb = small.tile([P, TILES], mybir.dt.float32)
            sums = small.tile([P, TILES], mybir.dt.float32)
            m = small.tile([P, TILES, WIDTH], mybir.dt.bfloat16)
            gsum = small.tile([P, TILES], mybir.dt.float32)
            junk = small.tile([P, C], mybir.dt.bfloat16)
            junk1 = small.tile([P, 1], mybir.dt.float32)

            def process(j0, pt, nt, na):
                """exp+sums+bands for nt tiles at global column j0, held in
                pt[:, :nt, :]. na trailing columns use the ACT accumulator.
                nt must be a multiple of GB and j0 GB-aligned."""
                nb = nt - na
                # batched exp first so the DVE reduce starts early; HW
                # accumulator paired with a junk read (proven v8 pattern).
                et = exp_pool.tile([P, G, C], mybir.dt.bfloat16)
                nc.scalar.activation(out=et[:, :nb, :], in_=pt[:, :nb, :],
                                     func=_ACT.Exp, accum_out=junk1)
                nc.vector.reduce_sum(out=sums[:, j0:j0 + nb],
                                     in_=et[:, :nb, :],
                                     axis=mybir.AxisListType.X)
                for g in range(na):
                    j = j0 + nb + g
                    nc.scalar.activation(out=junk, in_=pt[:, nb + g, :],
                                         func=_ACT.Exp,
                                         accum_out=sums[:, j:j + 1])
                # windowed term (gpsimd tensor ops NaN on HW; DVE takes it)
                for b in range(nt // GB):
                    gi = (j0 + b * GB) // GB
                    s = band_starts[gi]
                    nc.vector.tensor_mul(
                        out=m[:, j0 + b * GB:j0 + (b + 1) * GB, :],
                        in0=pt[:, b * GB:(b + 1) * GB, s:s + WIDTH],
                        in1=W_sb[:, j0 + b * GB:j0 + (b + 1) * GB, :])
                nc.vector.reduce_sum(out=gsum[:, j0:j0 + nt],
                                     in_=m[:, j0:j0 + nt, :],
                                     axis=mybir.AxisListType.X)

            # group 0 split 4+4: first exp starts ~10us earlier than one
            # 1.5MB DMA would allow. W/wsum ride the sync ring behind it
            # (the stream has ~20us of slack; ACT is the critical engine).
            pt0a = pred_pool.tile([P, G, C], mybir.dt.bfloat16)
            pt0b = pred_pool.tile([P, G, C], mybir.dt.bfloat16)
            nc.sync.dma_start(out=pt0a[:, :GB, :], in_=pred_r[0][:, :GB, :])
            nc.sync.dma_start(out=pt0b[:, :GB, :], in_=pred_r[0][:, GB:, :])
            nc.sync.dma_start(
                out=W_sb,
                in_=wband.ap().rearrange("p (j w) -> p j w", w=WIDTH))
            nc.sync.dma_start(out=wsum_sb, in_=wsum_in.ap())
            process(0, pt0a, GB, 0)
            process(GB, pt0b, GB, 0)

            for jg in range(1, NG):
                pt = pred_pool.tile([P, G, C], mybir.dt.bfloat16)
                nc.sync.dma_start(out=pt, in_=pred_r[jg])
                process(jg * G, pt, G, NA[jg])

            # tail: ~1.5us
            lse = small.tile([P, TILES], mybir.dt.float32)
            loss = small.tile([P, TILES], mybir.dt.float32)
            part = small.tile([P, 1], mybir.dt.float32)
            nc.scalar.activation(out=lse, in_=sums, func=_ACT.Ln)
            nc.vector.tensor_mul(out=loss, in0=wsum_sb, in1=lse)
            nc.vector.tensor_sub(out=loss, in0=loss, in1=gsum)
            nc.vector.reduce_sum(out=part, in_=loss,
                                 axis=mybir.AxisListType.X)
            nc.scalar.dma_start(out=out.ap(), in_=part)
    nc.compile()
    return nc


def _get_nc(band_starts):
    key = tuple(band_starts)
    if key not in _NC_CACHE:
        _NC_CACHE[key] = _build(key)
    return _NC_CACHE[key]


def _band_starts(target):
    """Static per-band-group class-band starts, shared by all cores."""
    lo = np.full(NGB, 1 << 30, np.int64)
    hi = np.full(NGB, -1, np.int64)
    bpc = B // CORES
    for c in range(CORES):
        tg = np.sort(target[c * bpc:(c + 1) * bpc].reshape(-1))
        blocks = tg.reshape(NGB, GB * P)
        lo = np.minimum(lo, np.clip(blocks.min(axis=1) - 3, 0, C - 1))
        hi = np.maximum(hi, np.clip(blocks.max(axis=1) + 3, 0, C - 1))
    assert (hi - lo + 1).max() <= WIDTH, "band width exceeded"
    s = np.clip((lo + hi + 1 - WIDTH) // 2, 0, C - WIDTH)
    assert np.all((lo >= s) & (hi < s + WIDTH))
    return tuple(int(x) for x in s)


def _shard_inputs(pred, target, band_starts):
    bpc = B // CORES
    s_per_tile = np.asarray(band_starts, np.int64)[np.arange(TILES) // GB]
    i_idx = np.arange(WIDTH)
    in_maps = []
    for c in range(CORES):
        shard_pred = pred[c * bpc:(c + 1) * bpc].reshape(SHARD, C)
        tg = np.ascontiguousarray(
            target[c * bpc:(c + 1) * bpc].reshape(SHARD)).astype(np.int64)
        order = np.argsort(tg, kind="stable")
        # HBM row r = jg*(P*G) + p*G + g  <->  token order[(jg*G+g)*P + p]
        r = np.arange(SHARD)
        jgr, rem = np.divmod(r, P * G)
        pr, gr = np.divmod(rem, G)
        perm = order[(jgr * G + gr) * P + pr]
        pred_rows = np.ascontiguousarray(shard_pred[perm]).astype(BF16)
        # W[p, j, i] = g(|band_col - tgt|), exact reference weights
        tgt_pj = tg[order].reshape(TILES, P).T              # (P, TILES)
        cpos = s_per_tile[None, :, None] + i_idx[None, None, :]
        d = np.abs(cpos - tgt_pj[:, :, None])               # (P, TILES, W)
        W = _GVAL[np.minimum(d, 7)]                         # f32 exact
        wsum = W.sum(axis=2, dtype=np.float32)              # (P, TILES) f32
        in_maps.append({
            "pred": pred_rows,
            "wband": np.ascontiguousarray(W.astype(BF16).reshape(P, -1)),
            "wsum": np.ascontiguousarray(wsum),
        })
    return in_maps


def _run(pred, target, **kwargs):
    pred = np.asarray(pred)
    target = np.asarray(target)
    band_starts = _band_starts(target)
    nc = _get_nc(band_starts)
    return bass_utils.run_bass_kernel_spmd(
        nc, _shard_inputs(pred, target, band_starts),
        core_ids=list(range(CORES)), **kwargs)


def kernel(pred, target):
    res = _run(pred, target)
    total = sum(float(r["partial"].astype(np.float64).sum())
                for r in res.results)
    return np.asarray(total / (B * T), dtype=np.float32)
